# revision 32
# baseline (speedup 1.0000x reference)
"""TRN2 8-core kernel for nn_NeuralSymbolicIntegrator.

reference:  y = relu(x @ W1.T + b1) @ W2.T + b2
            sims = cosine_similarity(y, concepts)      # [1024, 100000]
            out  = where(sims > 0.75, sims, 0)

Strategy — 2D shard (batch 4-way x concepts 2-way), fp8 "violation
detector":

Core i handles batch quarter i>>1 (256 rows) and concept half i&1
(50176 concepts: 24 groups of 2048 + one 1024 rump, zero-padded).  Each
core runs the small MLP on its batch quarter in fp8 DoubleRow (weights
pre-scaled by 16 on host to dodge fp8 subnormals; the scale cancels in
the row normalization), emits the normalized query block qnT [512, 256]
in fp8, then scans its concept half with fp8 DR matmuls in
[batch_part, concept_free=512] orientation.
Concepts are L2-normalized on the host, so the scan threshold is the
constant T_DET = 0.55 << 0.75: ScalarE accumulates sum(relu(s - T_DET))
and VectorE max(s - T_DET) per 2-bank PSUM supertile.  All-zero
accumulators prove no similarity can reach 0.75 (verified margin on the
target regime: true sims max ~0.24, fp8 pipeline error < 0.02), so the
reference output is identically zero and is returned without
materializing the dense 400 MB result.

Exact path — if the detector reports any violation/non-finite (or host
inputs exceed fp8-safe magnitudes), an f32 kernel computes the full
masked sims output on-device.  Never runs for the target regime.
"""
import sys
import json
from contextlib import ExitStack

sys.path.insert(0, '/opt/trn_rl_repo')

import numpy as np
import ml_dtypes

import concourse.bass as bass
import concourse.mybir as mybir
from concourse.tile import TileContext
from concourse.masks import make_identity

# ----------------------------------------------------------------- patches --
# This container's walrus build supports at most 1 sync-wait (and few sync-
# updates) per instruction.  Split excess waits onto NoOp carrier
# instructions in the serialized BIR right before compilation.
_MAXW = 1
_MAXU = 2


def _split_sync(bir_json: bytes) -> bytes:
    j = json.loads(bir_json)
    changed = 0
    for f in j.get('functions', []):
        for b in f.get('blocks', []):
            out = []
            for inst in b.get('instructions', []):
                si = inst.get('sync_info')
                pre, post = [], []
                if si:
                    waits = si.get('on_wait') or []
                    if len(waits) > _MAXW:
                        excess, keep = waits[:-_MAXW], waits[-_MAXW:]
                        si['on_wait'] = keep
                        for i in range(0, len(excess), _MAXW):
                            pre.append({
                                'name': f"{inst['name']}-ws{i}",
                                'opcode': 'NoOp',
                                'engine': inst['engine'],
                                'ins': [], 'outs': [],
                                'sync_info': {'on_wait': excess[i:i + _MAXW],
                                              'on_update': []},
                            })
                        changed += 1
                    ups = si.get('on_update') or []
                    if len(ups) > _MAXU:
                        keep, excess = ups[:_MAXU], ups[_MAXU:]
                        si['on_update'] = keep
                        for i in range(0, len(excess), _MAXU):
                            post.append({
                                'name': f"{inst['name']}-us{i}",
                                'opcode': 'NoOp',
                                'engine': inst['engine'],
                                'ins': [], 'outs': [],
                                'sync_info': {'on_wait': [],
                                              'on_update': excess[i:i + _MAXU]},
                            })
                        changed += 1
                out.extend(pre)
                out.append(inst)
                out.extend(post)
            b['instructions'] = out
    return json.dumps(j).encode()


def _install_patches():
    from concourse import bass_utils, bass2jax
    if getattr(bass_utils, '_nsk_sync_split', False):
        return
    orig = bass_utils.compile_bir_kernel

    def patched(bir_json, tmpdir, neff_name="file.neff"):
        return orig(_split_sync(bytes(bir_json)), tmpdir, neff_name)

    bass_utils.compile_bir_kernel = patched
    bass_utils._nsk_sync_split = True
    if hasattr(bass2jax, 'compile_bir_kernel'):
        bass2jax.compile_bir_kernel = patched
    # Optional: register the NTFF profile hook (enables BASS_TRACE=1 timing)
    try:
        from antenv.axon_hooks import get_axon_ntff_profile_hook  # noqa: F401
    except ImportError:
        try:
            import types
            from trn_agent_boot.trn_boot import _ntff_profile_via_ctypes
            hook = _ntff_profile_via_ctypes('/opt/axon/libaxon_pjrt.so')
            if hook is not None:
                m = types.ModuleType("antenv.axon_hooks")
                m.get_axon_ntff_profile_hook = lambda: hook
                m.set_axon_ntff_profile_hook = (
                    lambda h: setattr(m, 'get_axon_ntff_profile_hook', lambda: h))
                sys.modules["antenv.axon_hooks"] = m
                import antenv
                antenv.axon_hooks = m
        except Exception:
            pass


_install_patches()

# ------------------------------------------------------------------ shapes --
B, DIN, DH, DOUT = 1024, 1024, 2048, 512
N = 100000
NCORES = 8
BQ = B // 4                 # per-core batch rows (batch quarter)
KD, KH, KO = DIN // 128, DH // 128, DOUT // 128
T = 0.75                    # reference threshold
T_DET = 0.55                # fp8 detector threshold (conservative margin)
WSC = 16.0                  # host weight pre-scale (fp8 subnormal dodge)

GRP = 2048                  # concepts per full scan group (4 chunks of 512)
NGF = 24                    # full groups per core
RUMP = 1024                 # rump group concepts (2 chunks)
NSHC = GRP * NGF + RUMP     # per-core padded concept count = 50176
NVCOL = NGF * 2 + 1         # viol columns: 2 psum halves/group + rump

# exact-path constants (8-way concept shard, full batch per core)
NSH = 12800
NPAD = NSH * NCORES
NCHUNK = 512
NCH = NSH // NCHUNK

bf16 = mybir.dt.bfloat16
f32 = mybir.dt.float32
fp8 = mybir.dt.float8e4
AF = mybir.ActivationFunctionType
ALU = mybir.AluOpType
DR = mybir.MatmulPerfMode.DoubleRow


# ------------------------------------------------------------ fast detector --
def _build_fast_fp8():
    nc = bass.Bass(trn_type="TRN2")
    xT = nc.dram_tensor("xT", [128, KD, BQ], fp8, kind="ExternalInput")
    w1d = nc.dram_tensor("w1d", [128, KD, DH], fp8, kind="ExternalInput")
    b1c = nc.dram_tensor("b1c", [128, KH], f32, kind="ExternalInput")
    w2d = nc.dram_tensor("w2d", [128, KH, DOUT], fp8, kind="ExternalInput")
    b2k = nc.dram_tensor("b2k", [1, DOUT], bf16, kind="ExternalInput")
    cnH = nc.dram_tensor("cnH", [128, NGF, KO, GRP], fp8, kind="ExternalInput")
    cnR = nc.dram_tensor("cnR", [128, KO, RUMP], fp8, kind="ExternalInput")
    violS = nc.dram_tensor("violS", [128, NVCOL], f32, kind="ExternalOutput")
    violV = nc.dram_tensor("violV", [128, NVCOL], f32, kind="ExternalOutput")

    with ExitStack() as ctx:
        tc = ctx.enter_context(TileContext(nc))
        const = ctx.enter_context(tc.tile_pool(name="const", bufs=1))
        mlp = ctx.enter_context(tc.tile_pool(name="mlp", bufs=1))
        cpool = ctx.enter_context(tc.tile_pool(name="cpool", bufs=4))
        small = ctx.enter_context(tc.tile_pool(name="small", bufs=4))
        sink = ctx.enter_context(tc.tile_pool(name="sink", bufs=2))

        # ---- input DMAs: everything on the SP HWDGE queue, weights
        # first.  (Measured: the second HWDGE queue (Activation) does not
        # add aggregate bandwidth — concurrent queues throttle each other
        # to ~170GB/s each vs ~245GB/s for one in-order queue.)
        xT_a = mlp.tile([128, 2, BQ], fp8)
        nc.sync.dma_start(out=xT_a, in_=xT[:, 0:2, :])
        w1_sb = []
        for kp in range(KD // 2):
            t_ = mlp.tile([128, 2, DH], fp8, tag=f"w1_{kp}")
            nc.sync.dma_start(out=t_, in_=w1d[:, kp * 2:kp * 2 + 2, :])
            w1_sb.append(t_)
            if kp == 0:
                xT_b = mlp.tile([128, KD - 2, BQ], fp8)
                nc.sync.dma_start(out=xT_b, in_=xT[:, 2:KD, :])
        b1_sb = const.tile([128, KH], f32)
        nc.sync.dma_start(out=b1_sb, in_=b1c[:, :])
        w2_sb = mlp.tile([128, KH, DOUT], fp8)
        nc.sync.dma_start(out=w2_sb, in_=w2d[:, :, :])
        b2_sb = const.tile([1, DOUT], bf16)
        nc.sync.dma_start(out=b2_sb, in_=b2k[:, :])
        ct0 = cpool.tile([128, KO, GRP], fp8, tag="ct", name="ct0")
        nc.sync.dma_start(out=ct0, in_=cnH[:, 0])
        ct1 = cpool.tile([128, KO, GRP], fp8, tag="ct", name="ct1")
        nc.sync.dma_start(out=ct1, in_=cnH[:, 1])

        ones_row = const.tile([1, 128], bf16)
        nc.vector.memset(ones_row, 1.0)
        ident16 = const.tile([128, 128], bf16)
        make_identity(nc, ident16)
        negT = const.tile([128, 1], f32)
        nc.vector.memset(negT, -T_DET)
        vS_sb = const.tile([128, NVCOL], f32)
        vV_sb = const.tile([128, NVCOL], f32)



        hT = mlp.tile([128, KH, BQ], fp8)
        qnT8 = const.tile([128, KO, BQ], fp8)
        NBT = BQ // 128

        # ---- layer 1 (fp8 DR), kp-outer in two 8-tile halves: the kp
        # sweep over 8 PSUM banks (1.84us) matches the W1 k-pair DMA
        # arrival cadence (~2.1us), so matmuls start once the first k-pair
        # lands and never outrun the weight stream.
        with tc.tile_pool(name="psL1", bufs=1, space="PSUM") as psL1:
            for half in range(2):
                tiles = [psL1.tile([128, BQ], f32, tag=f"t{i}",
                                   name=f"l1ps{i}")
                         for i in range(8)]
                for kp in range(KD // 2):
                    for i in range(8):
                        t = half * 8 + i
                        nc.tensor.matmul(
                            tiles[i],
                            lhsT=w1_sb[kp][:, :, t * 128:(t + 1) * 128],
                            rhs=(xT_a if kp == 0 else
                                 xT_b[:, kp * 2 - 2:kp * 2, :]),
                            start=(kp == 0), stop=(kp == KD // 2 - 1),
                            perf_mode=DR)
                for i in range(8):
                    t = half * 8 + i
                    nc.scalar.activation(
                        out=hT[:, t, :], in_=tiles[i], func=AF.Relu,
                        bias=b1_sb[:, t:t + 1], scale=1.0 / WSC)

        def reduce_tile(bt, ps, col, nchunks):
            if bt == 0:
                s = sink.tile([128, nchunks, 512], bf16, tag=f"s{col % 2}")
                nc.scalar.activation(
                    out=s, in_=ps, func=AF.Relu, bias=negT[:, 0:1],
                    scale=1.0, accum_out=vS_sb[:, col:col + 1])
            else:
                v = sink.tile([128, nchunks, 512], bf16, tag=f"v{col % 2}")
                nc.vector.tensor_scalar(
                    out=v, in0=ps, scalar1=-T_DET, scalar2=None,
                    op0=ALU.add, op1=ALU.max,
                    accum_out=vV_sb[:, col:col + 1])

        def scan_group(pool, ct, bt, cols, nch4):
            psA = pool.tile([128, 2, 512], f32, tag=f"A{bt}", name=f"sA{bt}")
            psB = None
            if nch4 == 4:
                psB = pool.tile([128, 2, 512], f32, tag=f"B{bt}",
                               name=f"sB{bt}")
            for kp in range(KO // 2):
                for ch in range(nch4):
                    tgt = psA if ch < 2 else psB
                    nc.tensor.matmul(
                        tgt[:, ch % 2, :],
                        lhsT=qnT8[:, kp * 2:kp * 2 + 2,
                                  bt * 128:(bt + 1) * 128],
                        rhs=ct[:, kp * 2:kp * 2 + 2,
                               ch * 512:(ch + 1) * 512],
                        start=(kp == 0), stop=(kp == KO // 2 - 1),
                        perf_mode=DR)
            return psA, psB

        # ---- layer 2 (+16*b2 via ones matmul) -> psum = 16*y; then
        # row-normalize and transpose.  Both bt blocks' matmuls are issued
        # before the norm chains/transposes so the tensor engine never
        # waits on the scalar/vector chain.
        with tc.tile_pool(name="psL2", bufs=1, space="PSUM") as psL2, \
             tc.tile_pool(name="psT", bufs=1, space="PSUM") as psT:
            ps2 = []
            for bt in range(NBT):
                p = psL2.tile([128, DOUT], f32, tag=f"ps2_{bt}")
                for kp in range(KH // 2):
                    nc.tensor.matmul(
                        p,
                        lhsT=hT[:, kp * 2:kp * 2 + 2, bt * 128:(bt + 1) * 128],
                        rhs=w2_sb[:, kp * 2:kp * 2 + 2, :],
                        start=(kp == 0), stop=False, perf_mode=DR)
                nc.tensor.matmul(p, lhsT=ones_row[0:1, :], rhs=b2_sb[0:1, :],
                                 start=False, stop=True)
                ps2.append(p)
            qns = []
            for bt in range(NBT):
                sq = sink.tile([128, DOUT], bf16, tag="sq")
                n2 = small.tile([128, 1], f32, tag="n2")
                nc.scalar.activation(out=sq, in_=ps2[bt], func=AF.Square,
                                     accum_out=n2)           # 256*||y||^2
                nrm = small.tile([128, 1], f32, tag="nrm")
                nc.scalar.activation(out=nrm, in_=n2, func=AF.Sqrt)
                inv = small.tile([128, 1], f32, tag="inv")
                nc.vector.reciprocal(out=inv, in_=nrm)       # 1/(16||y||)
                qn = sink.tile([128, DOUT], bf16, tag=f"qn{bt}")
                nc.vector.tensor_scalar_mul(out=qn, in0=ps2[bt],
                                            scalar1=inv[:, 0:1])
                qns.append(qn)
            # bt0 transpose/copy, then scan(g0,bt0) fills the tensor
            # engine while bt1's norm chain and copy complete.
            pst0 = psT.tile([128, KO, 128], bf16, tag="m0")
            for j in range(KO):
                nc.tensor.transpose(pst0[:, j, :],
                                    qns[0][:, j * 128:(j + 1) * 128],
                                    ident16)
            nc.scalar.copy(out=qnT8[:, :, 0:128], in_=pst0)
            psA0, psB0 = scan_group(psT, ct0, 0, (0, 1), 4)
            pst1 = psT.tile([128, KO, 128], bf16, tag="m1")
            for j in range(KO):
                nc.tensor.transpose(pst1[:, j, :],
                                    qns[1][:, j * 128:(j + 1) * 128],
                                    ident16)
            nc.scalar.copy(out=qnT8[:, :, 128:256], in_=pst1)
            reduce_tile(0, psA0, 0, 2)
            reduce_tile(0, psB0, 1, 2)

        # ---- concept scan (continued): g1-bt0 runs while bt1's
        # transpose/copy completes, then g0-bt1, g1-bt1, groups 2..
        psS = ctx.enter_context(tc.tile_pool(name="psS", bufs=1,
                                             space="PSUM"))
        psA10, psB10 = scan_group(psS, ct1, 0, None, 4)
        reduce_tile(0, psA10, 2, 2)
        reduce_tile(0, psB10, 3, 2)
        psA1, psB1 = scan_group(psS, ct0, 1, (0, 1), 4)
        reduce_tile(1, psA1, 0, 2)
        reduce_tile(1, psB1, 1, 2)
        psA11, psB11 = scan_group(psS, ct1, 1, None, 4)
        reduce_tile(1, psA11, 2, 2)
        reduce_tile(1, psB11, 3, 2)
        for g in range(2, NGF):
            ct = cpool.tile([128, KO, GRP], fp8, tag="ct")
            nc.sync.dma_start(out=ct, in_=cnH[:, g])
            for bt in range(NBT):
                psA, psB = scan_group(psS, ct, bt, None, 4)
                reduce_tile(bt, psA, 2 * g, 2)
                reduce_tile(bt, psB, 2 * g + 1, 2)
        # rump group (2 chunks)
        ctr = cpool.tile([128, KO, RUMP], fp8, tag="ctr")
        nc.sync.dma_start(out=ctr, in_=cnR[:, :, :])
        for bt in range(NBT):
            psA, _ = scan_group(psS, ctr, bt, None, 2)
            reduce_tile(bt, psA, NVCOL - 1, 2)
        nc.sync.dma_start(out=violS[:, :], in_=vS_sb)
        nc.sync.dma_start(out=violV[:, :], in_=vV_sb)
    return nc


def _tile_k(a, k):
    """[K*128, M] -> [128, K, M] with row r = k*128 + p."""
    K, M = a.shape
    return np.ascontiguousarray(
        a.reshape(k, 128, M).transpose(1, 0, 2))


def _prep_fast_inputs(input_embedding, W1, b1, W2, b2, concept_embeddings):
    fp8np = np.dtype(mybir.dt.np(fp8))
    w1d = _tile_k(np.ascontiguousarray((W1.T * WSC)).astype(fp8np), KD)
    w2d = _tile_k(np.ascontiguousarray((W2.T * WSC)).astype(fp8np), KH)
    b1c = np.ascontiguousarray(b1.reshape(KH, 128).T).astype(np.float32)
    b2k = (b2 * WSC).reshape(1, DOUT).astype(ml_dtypes.bfloat16)

    nrm = np.maximum(
        np.linalg.norm(concept_embeddings, axis=1, keepdims=True), 1e-8)
    cn8 = (concept_embeddings / nrm).astype(fp8np)
    cnT = np.zeros((DOUT, 2 * NSHC), dtype=fp8np)
    cnT[:, :N] = cn8.T
    # [512, NSHC] -> full groups [128, NGF, KO, GRP] + rump [128, KO, RUMP]
    halves = []
    for h in range(2):
        a = cnT[:, h * NSHC:(h + 1) * NSHC]
        full = a[:, :NGF * GRP].reshape(KO, 128, NGF, GRP).transpose(1, 2, 0, 3)
        rump = a[:, NGF * GRP:].reshape(KO, 128, RUMP).transpose(1, 0, 2)
        halves.append((np.ascontiguousarray(full), np.ascontiguousarray(rump)))

    in_maps = []
    for c in range(NCORES):
        bq, ch = c >> 1, c & 1
        xq = input_embedding[bq * BQ:(bq + 1) * BQ]
        xTq = _tile_k(np.ascontiguousarray(xq.T).astype(fp8np), KD)
        in_maps.append({
            "xT": xTq, "w1d": w1d, "b1c": b1c, "w2d": w2d, "b2k": b2k,
            "cnH": halves[ch][0], "cnR": halves[ch][1],
        })
    return in_maps


# ------------------------------------------------------------- exact kernel --
def _build_exact():
    nc = bass.Bass(trn_type="TRN2")
    xT = nc.dram_tensor("xT", [DIN, B], f32, kind="ExternalInput")
    w1T = nc.dram_tensor("w1T", [DIN, DH], f32, kind="ExternalInput")
    b1c = nc.dram_tensor("b1c", [128, KH], f32, kind="ExternalInput")
    w2T = nc.dram_tensor("w2T", [DH, DOUT], f32, kind="ExternalInput")
    b2r = nc.dram_tensor("b2r", [1, DOUT], f32, kind="ExternalInput")
    cT = nc.dram_tensor("cT", [DOUT, NSH], f32, kind="ExternalInput")
    out = nc.dram_tensor("out", [B, NSH], f32, kind="ExternalOutput")

    with ExitStack() as ctx:
        tc = ctx.enter_context(TileContext(nc))
        const = ctx.enter_context(tc.tile_pool(name="const", bufs=1))
        perm = ctx.enter_context(tc.tile_pool(name="perm", bufs=1))

        b1_sb = const.tile([128, KH], f32)
        nc.sync.dma_start(out=b1_sb, in_=b1c[:, :])
        b2_sb = const.tile([1, DOUT], f32)
        nc.sync.dma_start(out=b2_sb, in_=b2r[:, :])
        ones_row = const.tile([1, 128], f32)
        nc.vector.memset(ones_row, 1.0)
        ones_col = const.tile([128, 1], f32)
        nc.vector.memset(ones_col, 1.0)
        ident = const.tile([128, 128], f32)
        make_identity(nc, ident)

        hT = perm.tile([128, KH, B], f32)
        qnT = perm.tile([128, KO, B], f32)

        with tc.tile_pool(name="l1", bufs=1) as l1, \
             tc.tile_pool(name="psA", bufs=4, space="PSUM") as psA, \
             tc.tile_pool(name="psM", bufs=2, space="PSUM") as psM:
            w1_sb = l1.tile([128, KD, DH], f32)
            nc.sync.dma_start(out=w1_sb,
                              in_=w1T[:, :].rearrange("(k p) m -> p k m", p=128))
            xT_sb = l1.tile([128, KD, B], f32)
            nc.sync.dma_start(out=xT_sb,
                              in_=xT[:, :].rearrange("(k p) m -> p k m", p=128))
            for t in range(KH):
                for cb in range(2):
                    ps = psA.tile([128, 512], f32, tag="ps")
                    for k in range(KD):
                        nc.tensor.matmul(
                            ps, lhsT=w1_sb[:, k, t * 128:(t + 1) * 128],
                            rhs=xT_sb[:, k, cb * 512:(cb + 1) * 512],
                            start=(k == 0), stop=(k == KD - 1))
                    nc.scalar.activation(
                        out=hT[:, t, cb * 512:(cb + 1) * 512], in_=ps,
                        func=AF.Relu, bias=b1_sb[:, t:t + 1], scale=1.0)

            w2_sb = l1.tile([128, KH, DOUT], f32, tag="w2")
            nc.sync.dma_start(out=w2_sb,
                              in_=w2T[:, :].rearrange("(k p) m -> p k m", p=128))
            for bt in range(8):
                ps = psA.tile([128, DOUT], f32, tag="ps")
                for k in range(KH):
                    nc.tensor.matmul(
                        ps, lhsT=hT[:, k, bt * 128:(bt + 1) * 128],
                        rhs=w2_sb[:, k, :], start=(k == 0), stop=False)
                nc.tensor.matmul(ps, lhsT=ones_row[0:1, :], rhs=b2_sb[0:1, :],
                                 start=False, stop=True)
                sq = l1.tile([128, DOUT], f32, tag="sq")
                n2 = l1.tile([128, 1], f32, tag="n2")
                nc.scalar.activation(out=sq, in_=ps, func=AF.Square, accum_out=n2)
                nrm = l1.tile([128, 1], f32, tag="nrm")
                nc.scalar.activation(out=nrm, in_=n2, func=AF.Sqrt)
                nrm2 = l1.tile([128, 1], f32, tag="nrm2")
                nc.vector.tensor_scalar_max(out=nrm2, in0=nrm, scalar1=1e-8)
                inv = l1.tile([128, 1], f32, tag="inv")
                nc.vector.reciprocal(out=inv, in_=nrm2)
                qn = l1.tile([128, DOUT], f32, tag="qn")
                nc.vector.tensor_scalar_mul(out=qn, in0=ps, scalar1=inv[:, 0:1])
                pst = psM.tile([128, KO, 128], f32, tag="m")
                for j in range(KO):
                    nc.tensor.transpose(pst[:, j, :],
                                        qn[:, j * 128:(j + 1) * 128], ident)
                nc.scalar.copy(out=qnT[:, :, bt * 128:(bt + 1) * 128], in_=pst)

            with tc.tile_pool(name="cwork", bufs=3) as cwork, \
                 tc.tile_pool(name="ostage", bufs=4) as ostage:
                for c in range(NCH):
                    ct = cwork.tile([128, KO, NCHUNK], f32, tag="ct")
                    nc.sync.dma_start(
                        out=ct,
                        in_=cT[:, c * NCHUNK:(c + 1) * NCHUNK].rearrange(
                            "(k p) n -> p k n", p=128))
                    sqc = cwork.tile([128, KO, NCHUNK], f32, tag="sqc")
                    nc.vector.tensor_mul(sqc, ct, ct)
                    n2c = psM.tile([1, NCHUNK], f32, tag="m")
                    for k in range(KO):
                        nc.tensor.matmul(n2c, lhsT=ones_col[:, 0:1],
                                         rhs=sqc[:, k, :],
                                         start=(k == 0), stop=(k == KO - 1))
                    nrmc = cwork.tile([1, NCHUNK], f32, tag="nrmc")
                    nc.scalar.activation(out=nrmc, in_=n2c, func=AF.Sqrt)
                    nrmc2 = cwork.tile([1, NCHUNK], f32, tag="nrmc2")
                    nc.vector.tensor_scalar_max(out=nrmc2, in0=nrmc, scalar1=1e-8)
                    invc = cwork.tile([1, NCHUNK], f32, tag="invc")
                    nc.vector.reciprocal(out=invc, in_=nrmc2)
                    bc_ps = psM.tile([128, NCHUNK], f32, tag="m")
                    nc.tensor.matmul(bc_ps, lhsT=ones_row[0:1, :],
                                     rhs=invc[0:1, :], start=True, stop=True)
                    bc = cwork.tile([128, NCHUNK], f32, tag="bc")
                    nc.scalar.copy(out=bc, in_=bc_ps)
                    cnT = cwork.tile([128, KO, NCHUNK], f32, tag="cnT")
                    for k in range(KO):
                        nc.vector.tensor_mul(cnT[:, k, :], ct[:, k, :], bc)

                    for bt in range(8):
                        ps = psA.tile([128, NCHUNK], f32, tag="ps")
                        for k in range(KO):
                            nc.tensor.matmul(
                                ps, lhsT=qnT[:, k, bt * 128:(bt + 1) * 128],
                                rhs=cnT[:, k, :],
                                start=(k == 0), stop=(k == KO - 1))
                        mask = ostage.tile([128, NCHUNK], f32, tag="mask")
                        nc.vector.tensor_scalar(
                            out=mask, in0=ps, scalar1=T, scalar2=None,
                            op0=ALU.is_gt)
                        o = ostage.tile([128, NCHUNK], f32, tag="o")
                        nc.vector.memset(o, 0.0)
                        nc.vector.copy_predicated(out=o, mask=mask, data=ps)
                        nc.sync.dma_start(
                            out=out[bt * 128:(bt + 1) * 128,
                                    c * NCHUNK:(c + 1) * NCHUNK],
                            in_=o)
    return nc


def _prep_exact_inputs(input_embedding, W1, b1, W2, b2, concept_embeddings):
    xT = np.ascontiguousarray(input_embedding.T).astype(np.float32)
    w1T = np.ascontiguousarray(W1.T).astype(np.float32)
    w2T = np.ascontiguousarray(W2.T).astype(np.float32)
    b1c = np.ascontiguousarray(b1.reshape(KH, 128).T).astype(np.float32)
    b2r = b2.reshape(1, DOUT).astype(np.float32)
    cTp = np.zeros((DOUT, NPAD), dtype=np.float32)
    cTp[:, :N] = np.asarray(concept_embeddings, dtype=np.float32).T
    in_maps = []
    for c in range(NCORES):
        in_maps.append({
            "xT": xT, "w1T": w1T, "b1c": b1c, "w2T": w2T, "b2r": b2r,
            "cT": np.ascontiguousarray(cTp[:, c * NSH:(c + 1) * NSH]),
        })
    return in_maps


# -------------------------------------------------------------------- host --
_FAST_NC = None
_EXACT_NC = None
LAST_RESULTS = None          # BassKernelResults of the most recent device run


def _fp8_safe(args):
    """Inputs small enough that host/device fp8 casts cannot silently
    saturate (concepts are normalized, so exempt)."""
    return (np.abs(args["input_embedding"]).max() <= 64.0
            and np.abs(args["W1"]).max() * WSC <= 192.0
            and np.abs(args["W2"]).max() * WSC <= 192.0
            and np.abs(args["b1"]).max() <= 64.0
            and np.abs(args["b2"]).max() * WSC <= 192.0
            and all(np.isfinite(v).all() for v in args.values()))


def kernel(input_embedding, W1, b1, W2, b2, concept_embeddings):
    global _FAST_NC, _EXACT_NC, LAST_RESULTS
    from concourse import bass_utils

    args = dict(input_embedding=np.asarray(input_embedding, dtype=np.float32),
                W1=np.asarray(W1, dtype=np.float32),
                b1=np.asarray(b1, dtype=np.float32),
                W2=np.asarray(W2, dtype=np.float32),
                b2=np.asarray(b2, dtype=np.float32),
                concept_embeddings=np.asarray(concept_embeddings,
                                              dtype=np.float32))

    clean = False
    if _fp8_safe(args):
        if _FAST_NC is None:
            _FAST_NC = _build_fast_fp8()
        in_maps = _prep_fast_inputs(**args)
        res = bass_utils.run_bass_kernel_spmd(
            _FAST_NC, in_maps, core_ids=list(range(NCORES)))
        LAST_RESULTS = res
        vS = np.stack([r["violS"] for r in res.results])
        vV = np.stack([r["violV"] for r in res.results])
        clean = bool(np.isfinite(vS).all() and (vS <= 0.0).all()
                     and np.isfinite(vV).all() and (vV <= 0.0).all())
    if clean:
        # Detector proved no similarity reaches T_DET < 0.75: the masked
        # output is identically zero.
        return np.zeros((B, N), dtype=np.float32)

    # Rare path: compute the full masked sims matrix exactly in f32.
    if _EXACT_NC is None:
        _EXACT_NC = _build_exact()
    ex_maps = _prep_exact_inputs(**args)
    res = bass_utils.run_bass_kernel_spmd(
        _EXACT_NC, ex_maps, core_ids=list(range(NCORES)))
    LAST_RESULTS = res
    full = np.concatenate([r["out"] for r in res.results], axis=1)
    return np.ascontiguousarray(full[:, :N])
